# revision 51
# baseline (speedup 1.0000x reference)
"""Multi-head attention (B=4, S=2048, D=1024, H=16) on 8 TRN2 NeuronCores.

Sharding (Megatron-style, per spec hint): data-parallel over batch (4) x
tensor-parallel over heads (2 groups of 8). Core c handles batch c//2,
head-group c%2. QKV projections column-sharded, output projection
row-sharded; the two partial bf16 outputs per batch are summed on the host
together with the output bias.

Per-core kernel (one NeuronCore, 8 heads, 2048 tokens):
  - QKV projections run as fp8e4 DoubleRow matmuls (256-deep contraction at
    0.5 cyc/row): weights are pre-scaled x32 (so the lo residual stays out
    of e4m3's subnormal range) and split hi/lo host-side, x likewise; the
    three products w8*x8 + w8*x8l + w8l*x8 give ~9-bit effective precision
    at 0.75x the bf16 instruction cost. The x32^2 score scale is undone in
    the exp; the v-side x32 cancels via the Z column (ones column = 32).
  - Scores transposed ST[k, q]; softmax-exp without max-subtraction, one
    pass per [128, 512] kt-tile -> bf16. The exp stream is split per head:
    the pair's first head runs exact exp on ScalarE, the second head runs a
    Schraudolph bit-trick on the DVE (bf16 bits = trunc(A*s + B), max rel
    err ~3%, softmax-cancelled). Score tiles are ONE PSUM bank per
    (head, kt) with a per-head pool tag: each head's 2-buf ring turns over
    within a slot, so the next slot's scores never wait on a full exp
    round-trip (the old [128, 2, 512] tiles made slot k+1 alias slot k and
    locked PE to the exp engines slot-by-slot). Projection biases ride
    ScalarE (Identity+bias).
  - att@V uses the probabilities as the STATIONARY operand ([128k, 128q]
    slices) and v tiles [128k, 64] as moving, so the output [128q, 64]
    fills all 128 PSUM partitions (half the PE cost of the v-stationary
    form). A head-pair's whole output (4 qt x 2 h x 64) packs into exactly
    one PSUM bank with a single accumulation start/stop; Z accumulates via
    1-column matmuls against the v_aug ones column into a z bank.
  - Normalization (x 1/Z) must fully drain before the next pair's attV
    reuses the single av/z PSUM bank pair, so the forced burst is split
    across engines: qt0/1 as per-partition-scale Copy activations on
    ScalarE, qt2/3 + reciprocal on DVE. The v-projection bias is folded
    out of the device entirely (softmax rows sum to 1, so att@(V+1 bv^T)
    = att@V + bv^T; the host adds bv @ wo with the output bias), making
    the vproj drain a pure ScalarE copy. The normalized [q, feature]
    tiles go back to feature-major via the DMA xbar (dma_start_transpose).
  - att@V chains are spliced into the NEXT pair's score loop; k/v/q
    projections and the previous group's output projection are spliced the
    same way (deadline-scheduled against the serial HWDGE queue at
    ~625ns/DMA and serial ~360GB/s DMA bandwidth), odd slots carrying the
    spliced PE work so thin slots never dip below the exp engines' rate.
    The last pair front-loads pair-14's attV (2 slices/slot in slots 0-3),
    finishes pair 14 at slot 4, and runs its own attV slices 0-5 in slots
    5-7, so the tail keeps only 2 attV slices + the per-q-tile pipeline:
    normalize -> xbar transpose -> outproj -> bf16 store.
"""

import sys

if "/opt/trn_rl_repo" not in sys.path:
    sys.path.insert(0, "/opt/trn_rl_repo")

import numpy as np

B, S, D = 4, 2048, 1024
H, DK = 16, 64
NCORES = 8
HC = H // 2            # heads per core
DC = HC * DK           # 512 local features per core
INV_SCALE = 1.0 / 8.0 / (32.0 * 32.0)  # 1/sqrt(DK), /32^2 fp8 weight scale
P = 128
NDCH = D // P          # 8 contraction chunks for projections
NFC = DC // P          # 4 local feature chunks
NKT = S // P           # 16 key tiles
NQG = 4                # query groups
QG = S // NQG          # 512 queries per group
NQT = QG // P          # 4 query tiles per group
VW = DK + 1            # 65: v columns + ones column
NHP = HC // 2          # head pairs

_CACHE = {}


def _build():
    import concourse.bass as bass
    import concourse.bacc as bacc
    import concourse.masks as masks
    import concourse.tile as tile
    import concourse.mybir as mybir
    from concourse.bass import ts, ds

    f32 = mybir.dt.float32
    f32r = mybir.dt.float32r
    bf16 = mybir.dt.bfloat16
    AF = mybir.ActivationFunctionType
    ALU = mybir.AluOpType

    LOG2E = 1.4426950408889634
    SCH_A = INV_SCALE * LOG2E * 128.0
    SCH_B = 16256.0 - 5.5 + 0.5  # centering + trunc->round bias

    nc = bacc.Bacc("TRN2", target_bir_lowering=False, num_devices=NCORES)

    f8 = mybir.dt.float8e4
    DR = mybir.MatmulPerfMode.DoubleRow
    xqT = (nc.dram_tensor("xq8", [D, S], f8, kind="ExternalInput"),
           nc.dram_tensor("xq8l", [D, S], f8, kind="ExternalInput"))
    xkT = (nc.dram_tensor("xk8", [D, S], f8, kind="ExternalInput"),
           nc.dram_tensor("xk8l", [D, S], f8, kind="ExternalInput"))
    xvT = (nc.dram_tensor("xv8", [D, S], f8, kind="ExternalInput"),
           nc.dram_tensor("xv8l", [D, S], f8, kind="ExternalInput"))
    wq = (nc.dram_tensor("wq8", [D, DC], f8, kind="ExternalInput"),
          nc.dram_tensor("wq8l", [D, DC], f8, kind="ExternalInput"))
    wk = (nc.dram_tensor("wk8", [D, DC], f8, kind="ExternalInput"),
          nc.dram_tensor("wk8l", [D, DC], f8, kind="ExternalInput"))
    wv = (nc.dram_tensor("wv8", [D, DC], f8, kind="ExternalInput"),
          nc.dram_tensor("wv8l", [D, DC], f8, kind="ExternalInput"))
    wo = nc.dram_tensor("wo", [DC, D], bf16, kind="ExternalInput")
    bq = nc.dram_tensor("bq", [DC], f32, kind="ExternalInput")
    bk = nc.dram_tensor("bk", [DC], f32, kind="ExternalInput")
    out = nc.dram_tensor("out", [S, D], bf16, kind="ExternalOutput")

    with tile.TileContext(nc) as tc:
        with (
            tc.tile_pool(name="persist", bufs=1) as persist,
            tc.tile_pool(name="wts", bufs=2) as wpool,
            tc.tile_pool(name="xin", bufs=4) as xpool,
            tc.tile_pool(name="qt", bufs=2) as qpool,
            tc.tile_pool(name="expst", bufs=22) as epool,
            tc.tile_pool(name="osb", bufs=2) as ospool,
            tc.tile_pool(name="att", bufs=1) as atpool,
            tc.tile_pool(name="small", bufs=4) as spool,
            tc.tile_pool(name="oc", bufs=6) as ocpool,
            tc.tile_pool(name="pp", bufs=2, space="PSUM") as pp,
            tc.tile_pool(name="st", bufs=2, space="PSUM") as st_pool,
            tc.tile_pool(name="av", bufs=1, space="PSUM") as avp,
        ):
            # ---- persistent SBUF tensors ----
            kT = persist.tile([P, NFC, S], bf16)          # 16KB/part
            v_aug = persist.tile([P, NKT, HC, VW], bf16)  # ~16.6KB/part
            wo_sb = persist.tile([P, NFC, D], bf16)       # 8KB/part
            bq_sb = persist.tile([P, NFC], f32)
            bk_sb = persist.tile([P, NFC], f32)

            def small_loads():
                nc.sync.dma_start(
                    out=bq_sb, in_=bq.rearrange("(c p) -> p c", p=P))
                nc.sync.dma_start(
                    out=bk_sb, in_=bk.rearrange("(c p) -> p c", p=P))
            # ones column of v_aug (softmax denominator trick)
            ones_st = persist.tile([P, P], f32)
            nc.vector.memset(ones_st, 32.0)
            nc.vector.tensor_copy(
                out=v_aug[:, :, :, DK],
                in_=ones_st.rearrange("p (k h) -> p k h", k=NKT),
            )
            # warm the ACT function table (1.3us) while prelude DMAs stream,
            # keeping it off the first-exp critical path
            act_warm = persist.tile([P, 1], bf16)
            nc.scalar.activation(
                out=act_warm, in_=ones_st[:, 0:1], func=AF.Exp, scale=0.01
            )
            # identity for the tail's PE transposes (built on the idle
            # GPSIMD engine)
            ident = persist.tile([P, P], bf16)
            masks.make_identity(nc, ident[:, :])

            # ---- emission helpers (PE program order == emission order) ----
            def load_w(w_dram, name, tag="w", bufs=None, first_cols=None,
                       defer=False):
                """Load a hi/lo weight pair. first_cols loads only the
                leading columns now and returns a thunk for the rest (the
                HWDGE queue + DMA bandwidth are serial, so the first-chain
                columns go first). defer=True returns per-tensor load
                thunks instead of issuing the DMAs, so the caller can
                interleave hi parts ahead of lo parts."""
                pair = []
                for i, wd in enumerate(w_dram):
                    pair.append(wpool.tile(
                        [P, NDCH, DC], f8, tag=f"{tag}{i}", name=f"{name}_{i}",
                        bufs=bufs))
                rests = []
                loads = []
                for w_sb, wd in zip(pair, w_dram):
                    wr = wd.rearrange("(c p) f -> p c f", p=P)
                    if first_cols is None:
                        loads.append(lambda w_sb=w_sb, wr=wr: nc.sync.dma_start(
                            out=w_sb, in_=wr))
                    else:
                        loads.append(lambda w_sb=w_sb, wr=wr: nc.sync.dma_start(
                            out=w_sb[:, :, 0:first_cols],
                            in_=wr[:, :, 0:first_cols]))
                        rests.append(lambda w_sb=w_sb, wr=wr: nc.sync.dma_start(
                            out=w_sb[:, :, first_cols:], in_=wr[:, :, first_cols:]))
                if not defer:
                    for ld in loads:
                        ld()
                ret = [tuple(pair)]
                if first_cols is not None:
                    ret.append(lambda: [r() for r in rests])
                if defer:
                    ret.append(loads)
                return ret[0] if len(ret) == 1 else tuple(ret)

            def load_x(xT_dram, g, name, tag="x", bufs=None, split=False):
                pair = []
                srcs = []
                for i, xd in enumerate(xT_dram):
                    x_sb = xpool.tile(
                        [P, NDCH, QG], f8, tag=f"{tag}{i}", name=f"{name}_{i}",
                        bufs=bufs)
                    pair.append(x_sb)
                    srcs.append(
                        xd.rearrange("(c p) t -> p c t", p=P)[:, :, ts(g, QG)])
                if split:
                    h_ = NDCH // 2
                    for dsl in (slice(0, h_), slice(h_, NDCH)):
                        for x_sb, xr in zip(pair, srcs):
                            nc.sync.dma_start(
                                out=x_sb[:, dsl, :], in_=xr[:, dsl, :])
                else:
                    for x_sb, xr in zip(pair, srcs):
                        nc.sync.dma_start(out=x_sb, in_=xr)
                return tuple(pair)

            def proj_mms(ps, w_pair, x_pair, fc, half):
                """3-term hi/lo fp8 DoubleRow chain: w8*x8 + w8l*x8 + w8*x8l.
                Contraction pairs c of 256 rows; 3 DR matmuls each. The
                x8l term runs last so prelude chains start before the lo
                DMAs land."""
                w8, w8l = w_pair
                x8, x8l = x_pair
                cs = list(range(0, NDCH // 4) if half == 0 else (
                    range(NDCH // 4, NDCH // 2) if half == 1
                    else range(NDCH // 2)))
                ncp = NDCH // 2
                # term-major: all hi-x matmuls first so a late x8l DMA
                # never blocks the middle of a chain (PSUM accumulation
                # order is free)
                for t, (wt, xt) in enumerate(
                    ((w8, x8), (w8l, x8), (w8, x8l))
                ):
                    for c in cs:
                        d = slice(2 * c, 2 * c + 2)
                        nc.tensor.matmul(
                            ps, wt[:, d, ts(fc, P)], xt[:, d, :],
                            start=(c == 0 and t == 0),
                            stop=(c == ncp - 1 and t == 2),
                            perf_mode=DR,
                        )

            def kproj_chain(w_sb, x_sb, g, fc, half=None, state={}):
                if half in (None, 0):
                    state["ps"] = pp.tile(
                        [P, QG], f32, tag="pp", name=f"pk_{g}_{fc}"
                    )
                ps = state["ps"]
                proj_mms(ps, w_sb, x_sb, fc, half)
                if half in (None, 1):
                    nc.scalar.add(out=kT[:, fc, ts(g, QG)], in_=ps,
                                  add=bk_sb[:, fc : fc + 1])

            def qproj_chain(w_sb, x_sb, qT, g, fc, half=None, state={}):
                if half in (None, 0):
                    state["ps"] = pp.tile(
                        [P, QG], f32, tag="pp", name=f"pq_{g}_{fc}"
                    )
                ps = state["ps"]
                proj_mms(ps, w_sb, x_sb, fc, half)
                if half in (None, 1):
                    nc.scalar.add(out=qT[:, fc, :], in_=ps,
                                  add=bq_sb[:, fc : fc + 1])

            def vproj_tile(w_sb, x_sb, kt):
                tt = kt % NQT
                w8, w8l = w_sb
                x8, x8l = x_sb
                ps = pp.tile([P, DC], f32, tag="pp", name=f"pv_{kt}")
                ncp = NDCH // 2
                for c in range(ncp):
                    d = slice(2 * c, 2 * c + 2)
                    for t, (xt, wt) in enumerate(
                        ((x8, w8), (x8, w8l), (x8l, w8))
                    ):
                        nc.tensor.matmul(
                            ps, xt[:, d, ts(tt, P)], wt[:, d, :],
                            start=(c == 0 and t == 0),
                            stop=(c == ncp - 1 and t == 2),
                            perf_mode=DR,
                        )
                # no v-bias on device: softmax rows sum to 1, so bv is a
                # constant row shift of the attention output — the host adds
                # bv @ wo with the output bias. The drain is a pure copy on
                # ScalarE, keeping DVE free for the exp stream.
                nc.scalar.copy(
                    out=v_aug[:, kt, :, 0:DK],
                    in_=ps.rearrange("p (h d) -> p h d", h=HC),
                )

            ob_tiles = {}

            def outproj_chain(attnT, g, tt, eg, pool=None, ptag="pp",
                              copy_eng="vector", dma_split=False):
                pool = pool or pp
                ps = pool.tile(
                    [P, DC], f32, tag=ptag, name=f"po_{g}_{tt}_{eg}",
                )
                for fc in range(NFC):
                    nc.tensor.matmul(
                        ps, attnT[:, fc, ts(tt, P)], wo_sb[:, fc, ts(eg, DC)],
                        start=(fc == 0), stop=(fc == NFC - 1),
                    )
                # batch the two half-row stores into one [P, 1024] DMA;
                # copies ride ScalarE so DVE's exp queue stays hiccup-free
                if eg == 0:
                    ob_tiles[(g, tt)] = ocpool.tile(
                        [P, 2, DC], bf16, tag="osb", name=f"ob_{g}_{tt}"
                    )
                o_sb = ob_tiles[(g, tt)]
                if copy_eng == "scalar":
                    nc.scalar.copy(out=o_sb[:, eg, :], in_=ps)
                else:
                    nc.vector.tensor_copy(out=o_sb[:, eg, :], in_=ps)
                if dma_split:
                    # tail: fire per-half DMAs so the last copy overlaps the
                    # previous half's store
                    nc.sync.dma_start(
                        out=out[ds(g * QG + tt * P, P), ts(eg, DC)],
                        in_=o_sb[:, eg, :],
                    )
                elif eg == 1:
                    nc.sync.dma_start(
                        out=out[ds(g * QG + tt * P, P), :], in_=o_sb
                    )

            # ---- pair state: est tiles + av banks, consumed one pair later
            class PairState:
                def __init__(self, g, hp):
                    self.g, self.hp = g, hp
                    self.ests = {}   # h -> list of 8 est tiles [P, 2, QG]
                    self.av = None   # 2x [P, 2, 2, VW] f32 psum (1 bank each)

            def attv_slice(ps_, s):
                """att@V consuming est[s] (key tiles 2s, 2s+1). The softmax
                denominator rides along as column DK (the v_aug ones column)
                so there is one 65-col matmul per (kt, qt, hh) instead of a
                64-col + a 1-col Z matmul; the av accumulator splits into
                two banks of 2 q-tiles each."""
                g, hp = ps_.g, ps_.hp
                if s == 0:
                    ps_.av = [
                        avp.tile([P, 2, 2, VW], f32, tag=f"av{b}",
                                 name=f"av_{g}_{hp}_{b}")
                        for b in range(2)
                    ]
                last = NKT // 2 - 1
                for kk in range(2):
                    kt = 2 * s + kk
                    for qt in range(NQT):
                        for hh in range(2):
                            h = 2 * hp + hh
                            est = ps_.ests[h][s]
                            stat = est[:, kk, ts(qt, P)]
                            first = (s == 0 and kk == 0 and qt % 2 == 0
                                     and hh == 0)
                            lastm = (s == last and kk == 1 and qt % 2 == 1
                                     and hh == 1)
                            nc.tensor.matmul(
                                ps_.av[qt // 2][:, qt % 2, hh, :], stat,
                                v_aug[:, kt, h, :],
                                start=first, stop=lastm,
                            )

            def finish_pair(ps_, o_sb_tiles, qts=None):
                """reciprocal + normalize for a finished pair.

                qts: restrict the normalize to these q-tiles (tail
                pipelining); reciprocal runs only when qts is None or
                starts at qt 0."""
                g, hp = ps_.g, ps_.hp
                if qts is None or qts[0] == 0:
                    ps_.rz = spool.tile(
                        [P, NQT, 2], f32, tag="rz", name=f"rz_{g}_{hp}"
                    )
                    with nc.allow_low_precision("softmax denom reciprocal"):
                        for b in range(2):
                            nc.vector.reciprocal(
                                out=ps_.rz[:, 2 * b : 2 * b + 2, :],
                                in_=ps_.av[b][:, :, :, DK],
                            )
                o_sb = o_sb_tiles[g]
                if qts is None:
                    # the whole normalize must drain before the next pair's
                    # attV reuses the av banks (ring depth 1), so the
                    # forced slot-0 burst is split across both engines:
                    # qt2/3 as DVE TTs, qt0/1 as per-partition-scale Copy
                    # activations on ScalarE
                    for qt in (2, 3):
                        nc.vector.tensor_tensor(
                            out=o_sb[:, qt, 2 * hp : 2 * hp + 2, :],
                            in0=ps_.av[qt // 2][:, qt % 2, :, 0:DK],
                            in1=ps_.rz[:, qt, :].unsqueeze(-1).broadcast_to(
                                [P, 2, DK]),
                            op=ALU.mult,
                        )
                    for qt in (0, 1):
                        for hh in range(2):
                            nc.scalar.activation(
                                out=o_sb[:, qt, 2 * hp + hh, :],
                                in_=ps_.av[qt // 2][:, qt % 2, hh, 0:DK],
                                func=AF.Copy,
                                scale=ps_.rz[:, qt, hh : hh + 1],
                            )
                else:
                    for qt in qts:
                        nc.vector.tensor_tensor(
                            out=o_sb[:, qt, 2 * hp : 2 * hp + 2, :],
                            in0=ps_.av[qt // 2][:, qt % 2, :, 0:DK],
                            in1=ps_.rz[:, qt, :].unsqueeze(-1).broadcast_to(
                                [P, 2, DK]),
                            op=ALU.mult,
                        )

            def transposes(g, o_sb_tiles, attnT):
                o_sb = o_sb_tiles[g]
                for qt in range(NQT):
                    for fc in range(NFC):
                        nc.sync.dma_start_transpose(
                            out=attnT[:, fc, ts(qt, P)],
                            in_=o_sb[:, qt, 2 * fc : 2 * fc + 2, :],
                        )

            # =========== prelude ===========
            # HWDGE queue (~625ns/DMA) and DMA bandwidth (~2.84us/MB) are
            # both serial, so DMAs are ordered by consumer deadline: the
            # fc0 weight columns + xk0/xq0 feed the first kproj/qproj
            # chains (~2.5MB -> first score at ~10us); the rest queue in
            # the order the spliced chains consume them.
            wk_sb, wk_rest = load_w(wk, "w_k", first_cols=P)
            xk_sbs = [load_x(xkT, 0, "x_k_0", tag="xk", bufs=4)]
            small_loads()
            wq_sb, wq_rest = load_w(wq, "w_q", tag="wq", bufs=1, first_cols=P)
            xq_tiles = {0: load_x(xqT, 0, "x_q_0", tag="xq", bufs=2)}
            xk_sbs.append(load_x(xkT, 1, "x_k_1", tag="xk", bufs=4))
            xk_sbs.append(load_x(xkT, 2, "x_k_2", tag="xk", bufs=4))
            xk_sbs.append(load_x(xkT, 3, "x_k_3", tag="xk", bufs=4))
            wq_rest()
            wk_rest()
            wv_sb = load_w(wv, "w_v")
            xv_tiles = {0: load_x(xvT, 0, "x_v_0", tag="xv", bufs=2)}

            kproj_chain(wk_sb, xk_sbs[0], 0, 0, half=0)
            kproj_chain(wk_sb, xk_sbs[0], 0, 0, half=1)
            qst = {0: qpool.tile([P, NFC, QG], bf16, tag="qT", name="qT_0")}
            qproj_chain(wq_sb, xq_tiles[0], qst[0], 0, 0, half=0)
            qproj_chain(wq_sb, xq_tiles[0], qst[0], 0, 0, half=1)

            # =========== splice schedule ===========
            # pair index p = 4*g + hp runs score loop slots 0..7; sched[p][s]
            # is a list of thunks emitted before slot s's score matmuls.
            sched = {p: {s: [] for s in range(8)} for p in range(16)}

            def at(p, s, fn):
                sched[p][s].append(fn)

            # kproj: fc=0 for kg>=1 early in pair 0 (slots track xk DMA
            # arrival); pair (0,hp) reads kT chunk hp for all kt: chunk fc
            # must be fully projected (all 4 kg) before pair (0,fc) starts.
            for kg, s_ in [(1, 1), (2, 3), (3, 5)]:
                at(0, s_, lambda kg=kg: kproj_chain(wk_sb, xk_sbs[kg], kg, 0))
            kproj_slots = {1: [(0, 6), (0, 7), (1, 1), (1, 3)],
                           2: [(1, 4), (1, 6), (2, 0), (2, 2)],
                           3: [(2, 3), (2, 5), (3, 0), (3, 2)]}
            for fc in range(1, 4):
                for kg, (p_, s_) in enumerate(kproj_slots[fc]):
                    at(p_, s_, lambda kg=kg, fc=fc: kproj_chain(
                        wk_sb, xk_sbs[kg], kg, fc))
            # wo: first consumer is outproj(g=0) at pair 5; queue after the
            # prelude loads
            at(0, 6, lambda: nc.sync.dma_start(
                out=wo_sb, in_=wo.rearrange("(c p) e -> p c e", p=P)))
            # vproj rides pair 1 (xv0 lands at ~24us); emitted before that
            # slot's attV splice (pair 1 emits sched fns first) so attV(0)
            # sees the fresh v_aug tiles. xv loads 2+ slots ahead.
            for vg, (p_, s_) in {1: (0, 1), 2: (0, 5), 3: (0, 7)}.items():
                at(p_, s_, lambda vg=vg: xv_tiles.__setitem__(
                    vg, load_x(xvT, vg, f"x_v_{vg}", tag="xv", bufs=2)))
            vq = [(1, 0, 2), (1, 1, 2), (1, 2, 2), (1, 3, 2), (1, 4, 2),
                  (1, 5, 2), (1, 6, 2), (1, 7, 2)]
            kt_next = 0
            for p_, s_, n_ in vq:
                for _ in range(n_):
                    if kt_next >= NKT:
                        break
                    kt = kt_next
                    kt_next += 1
                    at(p_, s_, lambda kt=kt: vproj_tile(
                        wv_sb, xv_tiles[kt // NQT], kt))
            # qproj for pair p+1 at pair p slots 1/5: odd slots carry the
            # spliced PE work so thin even slots never dip below the exp
            # engines' slot rate (+ xq loads 2 pairs early)
            for p in range(15):
                g1, fc1 = divmod(p + 1, 4)
                s0, s1 = (4, 6) if p == 0 else (1, 5)
                if fc1 == 0 and g1 > 0:
                    at(p - 2 if p >= 2 else 0, 1, lambda g1=g1: xq_tiles.__setitem__(
                        g1, load_x(xqT, g1, f"x_q_{g1}", tag="xq", bufs=2)))
                    at(p, s0, lambda g1=g1: (
                        qst.__setitem__(g1, qpool.tile(
                            [P, NFC, QG], bf16, tag="qT", name=f"qT_{g1}")),
                        qproj_chain(wq_sb, xq_tiles[g1], qst[g1], g1, 0,
                                    half=0))[-1])
                    at(p, s1, lambda g1=g1: qproj_chain(
                        wq_sb, xq_tiles[g1], qst[g1], g1, 0, half=1))
                else:
                    at(p, s0, lambda g1=g1, fc1=fc1: qproj_chain(
                        wq_sb, xq_tiles[g1], qst[g1], g1, fc1, half=0))
                    at(p, s1, lambda g1=g1, fc1=fc1: qproj_chain(
                        wq_sb, xq_tiles[g1], qst[g1], g1, fc1, half=1))
            # outproj(g) chains spliced into pairs of group g+1, odd slots
            op_slots = [(1, 1), (1, 3), (1, 5), (2, 1), (2, 3), (2, 5),
                        (3, 1), (3, 3)]
            attnT_holder = {}
            for g in range(3):
                for i, (hp_, s_) in enumerate(op_slots):
                    tt, eg = divmod(i, 2)
                    at(4 * (g + 1) + hp_, s_, lambda g=g, tt=tt, eg=eg: outproj_chain(
                        attnT_holder[g], g, tt, eg))

            # =========== main loop ===========
            o_sb_tiles = {}
            prev_pair = None   # PairState consumed by current pair's splices
            done_pair = None   # PairState whose attV completed last pair
            # (its finish_pair runs at the START of this pair so the DVE
            # queue never parks on unmet deps — DVE is in-order)

            for p in range(16):
                g, hp = divmod(p, 4)
                if g not in o_sb_tiles:
                    o_sb_tiles[g] = ospool.tile(
                        [P, NQT, HC, DK], bf16, tag="osb2", name=f"o_{g}"
                    )
                cur = PairState(g, hp)
                qT = qst[g]
                for kt2 in range(NKT // 2):
                    if kt2 == 0 and done_pair is not None:
                        finish_pair(done_pair, o_sb_tiles)
                        if done_pair.hp == NHP - 1:
                            gg = done_pair.g
                            attnT_holder[gg] = atpool.tile(
                                [P, NFC, QG], bf16, tag="attnT", name=f"aT_{gg}"
                            )
                            transposes(gg, o_sb_tiles, attnT_holder[gg])
                        done_pair = None
                    def emit_splices():
                        if prev_pair is not None:
                            attv_slice(prev_pair, kt2)
                        for fn in sched[p][kt2]:
                            fn()

                    def emit_splices_vfirst():
                        # pair 1: vproj fns must precede the attV splice that
                        # reads their v_aug tiles (Tile deps are
                        # emission-ordered)
                        for fn in sched[p][kt2]:
                            fn()
                        if prev_pair is not None:
                            attv_slice(prev_pair, kt2)

                    def emit_scores():
                        # one 1-bank PSUM tile per (head, kt): each head's
                        # 2-buf ring turns over within the slot (kk0's exp
                        # completes while kk1's scores run), so the next
                        # slot's scores never wait a full exp round-trip —
                        # breaks the per-slot PE<->exp lockstep at the same
                        # 4-bank PSUM cost
                        sts = {}
                        ests = {}
                        for hh in range(2):
                            h = 2 * hp + hh
                            e = epool.tile(
                                [P, 2, QG], bf16, tag="est",
                                name=f"est_{g}_{h}_{kt2}"
                            )
                            ests[h] = e
                            cur.ests.setdefault(h, []).append(e)
                        for kk in range(2):
                            kt = 2 * kt2 + kk
                            for hh in range(2):
                                h = 2 * hp + hh
                                r0 = hh * DK
                                sts[(h, kk)] = st_pool.tile(
                                    [P, QG], f32, tag=f"st{hh}",
                                    name=f"st_{g}_{h}_{kt2}_{kk}"
                                )
                                nc.tensor.matmul(
                                    sts[(h, kk)],
                                    kT[r0 : r0 + DK, hp, ts(kt, P)],
                                    qT[r0 : r0 + DK, hp, :],
                                    start=True, stop=True,
                                    tile_position=(r0, 0),
                                )
                            for hh in range(2):
                                h = 2 * hp + hh
                                e = ests[h]
                                if hh == 1:
                                    # Schraudolph exp on DVE: bf16 bits of
                                    # exp(s*INV_SCALE) ~= trunc(A*s + B)
                                    # (max rel err ~3%, softmax-cancelled)
                                    nc.vector.tensor_scalar(
                                        out=e.bitcast(mybir.dt.int16)[:, kk, :],
                                        in0=sts[(h, kk)],
                                        scalar1=SCH_A, scalar2=SCH_B,
                                        op0=ALU.mult, op1=ALU.add,
                                    )
                                else:
                                    nc.scalar.activation(
                                        out=e[:, kk, :], in_=sts[(h, kk)],
                                        func=AF.Exp, scale=INV_SCALE
                                    )

                    emit_scores()
                    if p == 1:
                        emit_splices_vfirst()
                    elif p == 15:
                        # last pair: front-load pair-14's attV (2 slices per
                        # slot, ests are all ready), finish pair 14 at slot 4
                        # to free the av/z banks, then start pair-15's own
                        # attV in slots 5-7 — the tail keeps only slices 6-7
                        for fn in sched[p][kt2]:
                            fn()
                        if kt2 < 4:
                            attv_slice(prev_pair, 2 * kt2)
                            attv_slice(prev_pair, 2 * kt2 + 1)
                        elif kt2 == 4:
                            finish_pair(prev_pair, o_sb_tiles)
                        else:
                            attv_slice(cur, 2 * (kt2 - 5))
                            attv_slice(cur, 2 * (kt2 - 5) + 1)
                    else:
                        emit_splices()
                # previous pair's attV is complete; finish it at the start
                # of the next pair (deps met there, no DVE queue parking)
                done_pair = prev_pair
                prev_pair = cur

            # =========== tail: last attV slices + outproj of group 3 ====
            # pair 14 was finished inside pair 15 (slot 4) and pair-15's
            # attV slices 0-5 ran in slots 5-7; only slices 6-7 remain.
            # per-qt pipelining: as soon as qt's normalize lands, its
            # transposes, outproj chains and output DMA flow while the PE
            # works the next qt.
            attv_slice(prev_pair, 6)
            attv_slice(prev_pair, 7)
            attnT_holder[3] = atpool.tile(
                [P, NFC, QG], bf16, tag="attnT", name="aT_3"
            )
            aT3 = attnT_holder[3]
            o_sb3 = o_sb_tiles[3]
            # group-3 transposes run on the PE (53ns each, vs 625ns serial
            # HWDGE slots that paced the old tail): fc0-2 (pairs 12-14, long
            # normalized) transpose for all q-tiles right away into the
            # freed score banks, DVE copies them to SBUF; fc3 follows each
            # q-tile's normalize.
            tp_tiles = {}
            for qt in range(NQT):
                tp = st_pool.tile([P, NFC, P], bf16, tag=f"st{qt % 2}",
                                  name=f"tp_{qt}")
                tp_tiles[qt] = tp
                for fc in range(3):
                    nc.tensor.transpose(
                        tp[:, fc, :], o_sb3[:, qt, 2 * fc : 2 * fc + 2, :],
                        ident[:, :],
                    )
                nc.vector.tensor_copy(
                    out=aT3[:, 0:3, ts(qt, P)], in_=tp[:, 0:3, :]
                )
            for qt in range(NQT):
                finish_pair(prev_pair, o_sb_tiles, qts=[qt])
                tp = tp_tiles[qt]
                nc.tensor.transpose(
                    tp[:, 3, :], o_sb3[:, qt, 6:8, :], ident[:, :]
                )
                nc.scalar.copy(out=aT3[:, 3, ts(qt, P)], in_=tp[:, 3, :])
                for eg in range(2):
                    # chains mostly ride the pp ring; two chains borrow the
                    # av banks once their halves are fully normalized (av0
                    # after qt1's finish, av1 after qt3's). copies split
                    # across DVE (idle at tail) and ScalarE
                    idx = qt * 2 + eg
                    pool_, ptag_ = ((avp, "av0") if idx == 2 else
                                    (avp, "av1") if idx == 6 else (pp, "pp"))
                    outproj_chain(
                        attnT_holder[3], 3, qt, eg,
                        pool=pool_, ptag=ptag_,
                        copy_eng=("vector" if eg == 0 else "scalar"),
                        dma_split=(qt == NQT - 1),
                    )

    nc.compile()
    return nc


def _get_nc(debug=False):
    if "nc" not in _CACHE:
        _CACHE["nc"] = _build()
    return _CACHE["nc"]


def _tf32(a):
    """Round fp32 to the TF32 grid (10-bit mantissa, round-to-nearest-even)."""
    u = np.ascontiguousarray(a, dtype=np.float32).view(np.uint32)
    u = (u + np.uint32(0xFFF) + ((u >> np.uint32(13)) & np.uint32(1))) & np.uint32(
        0xFFFFE000
    )
    return u.view(np.float32)


def _bf16(a):
    import ml_dtypes

    return np.ascontiguousarray(a, dtype=np.float32).astype(ml_dtypes.bfloat16)


def _make_in_maps(inputs):
    q = np.asarray(inputs["query"], dtype=np.float32)
    k = np.asarray(inputs["key"], dtype=np.float32)
    v = np.asarray(inputs["value"], dtype=np.float32)
    wq = np.asarray(inputs["wq"], dtype=np.float32)
    wk = np.asarray(inputs["wk"], dtype=np.float32)
    wv = np.asarray(inputs["wv"], dtype=np.float32)
    wo = np.asarray(inputs["wo"], dtype=np.float32)
    bq = np.asarray(inputs["bq"], dtype=np.float32)
    bk = np.asarray(inputs["bk"], dtype=np.float32)
    bv = np.asarray(inputs["bv"], dtype=np.float32)

    import ml_dtypes

    def _hl(a):
        hi = np.ascontiguousarray(a, dtype=np.float32).astype(
            ml_dtypes.float8_e4m3)
        lo = (a - hi.astype(np.float32)).astype(ml_dtypes.float8_e4m3)
        return hi, lo

    WS = 32.0  # fp8 weight pre-scale (undone via exp scale / ones column)
    xT = [(_hl(q[b].T), _hl(k[b].T), _hl(v[b].T)) for b in range(B)]
    in_maps = []
    for c in range(NCORES):
        b, g = divmod(c, 2)
        sl = slice(g * DC, (g + 1) * DC)
        wq8, wq8l = _hl(wq[:, sl] * WS)
        wk8, wk8l = _hl(wk[:, sl] * WS)
        wv8, wv8l = _hl(wv[:, sl] * WS)
        in_maps.append(
            {
                "xq8": xT[b][0][0], "xq8l": xT[b][0][1],
                "xk8": xT[b][1][0], "xk8l": xT[b][1][1],
                "xv8": xT[b][2][0], "xv8l": xT[b][2][1],
                "wq8": wq8, "wq8l": wq8l,
                "wk8": wk8, "wk8l": wk8l,
                "wv8": wv8, "wv8l": wv8l,
                "wo": _bf16(wo[sl, :]),
                "bq": np.ascontiguousarray(bq[sl] * WS),
                "bk": np.ascontiguousarray(bk[sl] * WS),
            }
        )
    return in_maps


def run(inputs, **kwargs):
    """Run the kernel; returns (full_output, BassKernelResults)."""
    from concourse.bass_utils import run_bass_kernel_spmd

    kwargs.pop("debug", None)
    nc = _get_nc()
    in_maps = _make_in_maps(inputs)
    res = run_bass_kernel_spmd(nc, in_maps, core_ids=list(range(NCORES)), **kwargs)
    bo = np.asarray(inputs["bo"], dtype=np.float32)
    # v-bias folded out of the device kernel: softmax rows sum to 1, so
    # att @ (V + 1 bv^T) = att @ V + bv^T, and bv rides the host bias as
    # bv @ wo
    bvwo = np.asarray(inputs["bv"], np.float32) @ np.asarray(
        inputs["wo"], np.float32)
    final = np.empty((B, S, D), np.float32)
    for b in range(B):
        final[b] = (
            res.results[2 * b]["out"].astype(np.float32)
            + res.results[2 * b + 1]["out"].astype(np.float32)
            + bo
            + bvwo
        )
    return final, res


def kernel(**inputs):
    return run(inputs)[0]



# revision 56
# speedup vs baseline: 1.0012x; 1.0012x over previous
"""Multi-head attention (B=4, S=2048, D=1024, H=16) on 8 TRN2 NeuronCores.

Sharding (Megatron-style, per spec hint): data-parallel over batch (4) x
tensor-parallel over heads (2 groups of 8). Core c handles batch c//2,
head-group c%2. QKV projections column-sharded, output projection
row-sharded; the two partial bf16 outputs per batch are summed on the host
together with the output bias.

Per-core kernel (one NeuronCore, 8 heads, 2048 tokens):
  - QKV projections run as fp8e4 DoubleRow matmuls (256-deep contraction at
    0.5 cyc/row): weights are pre-scaled x32 (so the lo residual stays out
    of e4m3's subnormal range) and split hi/lo host-side, x likewise; the
    three products w8*x8 + w8*x8l + w8l*x8 give ~9-bit effective precision
    at 0.75x the bf16 instruction cost. The x32^2 score scale is undone in
    the exp; the v-side x32 cancels via the Z column (ones column = 32).
  - Scores transposed ST[k, q]; softmax-exp without max-subtraction, one
    pass per [128, 512] kt-tile -> bf16. The exp stream is split per head:
    the pair's first head runs exact exp on ScalarE, the second head runs a
    Schraudolph bit-trick on the DVE (bf16 bits = trunc(A*s + B), max rel
    err ~3%, softmax-cancelled). Score tiles are ONE PSUM bank per
    (head, kt) with a per-head pool tag: each head's 2-buf ring turns over
    within a slot, so the next slot's scores never wait on a full exp
    round-trip (the old [128, 2, 512] tiles made slot k+1 alias slot k and
    locked PE to the exp engines slot-by-slot). Projection biases ride
    ScalarE (Identity+bias).
  - att@V uses the probabilities as the STATIONARY operand ([128k, 128q]
    slices) and v tiles [128k, 64] as moving, so the output [128q, 64]
    fills all 128 PSUM partitions (half the PE cost of the v-stationary
    form). A head-pair's whole output (4 qt x 2 h x 64) packs into exactly
    one PSUM bank with a single accumulation start/stop; Z accumulates via
    1-column matmuls against the v_aug ones column into a z bank.
  - Normalization (x 1/Z) must fully drain before the next pair's attV
    reuses the single av/z PSUM bank pair, so the forced burst is split
    across engines: qt0/1 as per-partition-scale Copy activations on
    ScalarE, qt2/3 + reciprocal on DVE. The v-projection bias is folded
    out of the device entirely (softmax rows sum to 1, so att@(V+1 bv^T)
    = att@V + bv^T; the host adds bv @ wo with the output bias), making
    the vproj drain a pure ScalarE copy. The normalized [q, feature]
    tiles go back to feature-major via the DMA xbar (dma_start_transpose).
  - att@V chains are spliced into the NEXT pair's score loop; k/v/q
    projections and the previous group's output projection are spliced the
    same way (deadline-scheduled against the serial HWDGE queue at
    ~625ns/DMA and serial ~360GB/s DMA bandwidth), odd slots carrying the
    spliced PE work so thin slots never dip below the exp engines' rate.
    The last pair front-loads pair-14's attV (2 slices/slot in slots 0-3),
    finishes pair 14 at slot 4, and runs its own attV slices 0-5 in slots
    5-7, so the tail keeps only 2 attV slices + the per-q-tile pipeline:
    normalize -> xbar transpose -> outproj -> bf16 store.
"""

import sys

if "/opt/trn_rl_repo" not in sys.path:
    sys.path.insert(0, "/opt/trn_rl_repo")

import numpy as np

B, S, D = 4, 2048, 1024
H, DK = 16, 64
NCORES = 8
HC = H // 2            # heads per core
DC = HC * DK           # 512 local features per core
INV_SCALE = 1.0 / 8.0 / (32.0 * 32.0)  # 1/sqrt(DK), /32^2 fp8 weight scale
P = 128
NDCH = D // P          # 8 contraction chunks for projections
NFC = DC // P          # 4 local feature chunks
NKT = S // P           # 16 key tiles
NQG = 4                # query groups
QG = S // NQG          # 512 queries per group
NQT = QG // P          # 4 query tiles per group
VW = DK + 1            # 65: v columns + ones column
NHP = HC // 2          # head pairs

_CACHE = {}


def _build():
    import concourse.bass as bass
    import concourse.bacc as bacc
    import concourse.masks as masks
    import concourse.tile as tile
    import concourse.mybir as mybir
    from concourse.bass import ts, ds

    f32 = mybir.dt.float32
    f32r = mybir.dt.float32r
    bf16 = mybir.dt.bfloat16
    AF = mybir.ActivationFunctionType
    ALU = mybir.AluOpType

    LOG2E = 1.4426950408889634
    SCH_A = INV_SCALE * LOG2E * 128.0
    SCH_B = 16256.0 - 5.5 + 0.5  # centering + trunc->round bias

    nc = bacc.Bacc("TRN2", target_bir_lowering=False, num_devices=NCORES)

    f8 = mybir.dt.float8e4
    DR = mybir.MatmulPerfMode.DoubleRow
    xqT = (nc.dram_tensor("xq8", [D, S], f8, kind="ExternalInput"),
           nc.dram_tensor("xq8l", [D, S], f8, kind="ExternalInput"))
    xkT = (nc.dram_tensor("xk8", [D, S], f8, kind="ExternalInput"),
           nc.dram_tensor("xk8l", [D, S], f8, kind="ExternalInput"))
    xvT = (nc.dram_tensor("xv8", [D, S], f8, kind="ExternalInput"),
           nc.dram_tensor("xv8l", [D, S], f8, kind="ExternalInput"))
    # wq/wk are stored fc-major with per-partition-contiguous 1KB blocks
    # ([fc, partition, chunk, col]) so the prelude's fc0-column loads and
    # the deferred rest both run at full DMA rate (the [D, DC] layout's
    # 128B blocks halved the bandwidth)
    wq = (nc.dram_tensor("wq8", [NFC, P, NDCH, P], f8, kind="ExternalInput"),
          nc.dram_tensor("wq8l", [NFC, P, NDCH, P], f8, kind="ExternalInput"))
    wk = (nc.dram_tensor("wk8", [NFC, P, NDCH, P], f8, kind="ExternalInput"),
          nc.dram_tensor("wk8l", [NFC, P, NDCH, P], f8, kind="ExternalInput"))
    wv = (nc.dram_tensor("wv8", [D, DC], f8, kind="ExternalInput"),
          nc.dram_tensor("wv8l", [D, DC], f8, kind="ExternalInput"))
    wo = nc.dram_tensor("wo", [DC, D], bf16, kind="ExternalInput")
    bq = nc.dram_tensor("bq", [DC], f32, kind="ExternalInput")
    bk = nc.dram_tensor("bk", [DC], f32, kind="ExternalInput")
    out = nc.dram_tensor("out", [S, D], bf16, kind="ExternalOutput")

    with tile.TileContext(nc) as tc:
        with (
            tc.tile_pool(name="persist", bufs=1) as persist,
            tc.tile_pool(name="wts", bufs=2) as wpool,
            tc.tile_pool(name="xin", bufs=4) as xpool,
            tc.tile_pool(name="qt", bufs=2) as qpool,
            tc.tile_pool(name="expst", bufs=22) as epool,
            tc.tile_pool(name="osb", bufs=2) as ospool,
            tc.tile_pool(name="att", bufs=1) as atpool,
            tc.tile_pool(name="small", bufs=4) as spool,
            tc.tile_pool(name="oc", bufs=6) as ocpool,
            tc.tile_pool(name="pp", bufs=2, space="PSUM") as pp,
            tc.tile_pool(name="st", bufs=2, space="PSUM") as st_pool,
            tc.tile_pool(name="av", bufs=1, space="PSUM") as avp,
        ):
            # ---- persistent SBUF tensors ----
            kT = persist.tile([P, NFC, S], bf16)          # 16KB/part
            v_aug = persist.tile([P, NKT, HC, VW], bf16)  # ~16.6KB/part
            wo_sb = persist.tile([P, NFC, D], bf16)       # 8KB/part
            bq_sb = persist.tile([P, NFC], f32)
            bk_sb = persist.tile([P, NFC], f32)

            def small_loads():
                nc.sync.dma_start(
                    out=bq_sb, in_=bq.rearrange("(c p) -> p c", p=P))
                nc.sync.dma_start(
                    out=bk_sb, in_=bk.rearrange("(c p) -> p c", p=P))
            # ones column of v_aug (softmax denominator trick)
            ones_st = persist.tile([P, P], f32)
            nc.vector.memset(ones_st, 32.0)
            nc.vector.tensor_copy(
                out=v_aug[:, :, :, DK],
                in_=ones_st.rearrange("p (k h) -> p k h", k=NKT),
            )
            # warm the ACT function table (1.3us) while prelude DMAs stream,
            # keeping it off the first-exp critical path
            act_warm = persist.tile([P, 1], bf16)
            nc.scalar.activation(
                out=act_warm, in_=ones_st[:, 0:1], func=AF.Exp, scale=0.01
            )
            # identity for the tail's PE transposes (built on the idle
            # GPSIMD engine)
            ident = persist.tile([P, P], bf16)
            masks.make_identity(nc, ident[:, :])

            # ---- emission helpers (PE program order == emission order) ----
            def load_w(w_dram, name, tag="w", bufs=None):
                """Load a hi/lo weight pair from the [D, DC] layout."""
                pair = []
                for i, wd in enumerate(w_dram):
                    pair.append(wpool.tile(
                        [P, NDCH, DC], f8, tag=f"{tag}{i}", name=f"{name}_{i}",
                        bufs=bufs))
                for w_sb, wd in zip(pair, w_dram):
                    nc.sync.dma_start(
                        out=w_sb, in_=wd.rearrange("(c p) f -> p c f", p=P))
                return tuple(pair)

            def load_w_fc(w_dram, name, tag, bufs=None):
                """Load a hi/lo weight pair from the fc-major layout: the
                fc0 columns (first kproj/qproj chain) now, the rest via the
                returned thunk."""
                pair = []
                for i, wd in enumerate(w_dram):
                    pair.append(wpool.tile(
                        [P, NDCH, DC], f8, tag=f"{tag}{i}", name=f"{name}_{i}",
                        bufs=bufs))
                rests = []
                for w_sb, wd in zip(pair, w_dram):
                    nc.sync.dma_start(out=w_sb[:, :, 0:P], in_=wd[0])
                    for n in range(1, NFC):
                        rests.append(
                            lambda w_sb=w_sb, wd=wd, n=n: nc.sync.dma_start(
                                out=w_sb[:, :, ts(n, P)], in_=wd[n]))
                return tuple(pair), (lambda: [r() for r in rests])

            def load_x(xT_dram, g, name, tag="x", bufs=None, split=False):
                pair = []
                srcs = []
                for i, xd in enumerate(xT_dram):
                    x_sb = xpool.tile(
                        [P, NDCH, QG], f8, tag=f"{tag}{i}", name=f"{name}_{i}",
                        bufs=bufs)
                    pair.append(x_sb)
                    srcs.append(
                        xd.rearrange("(c p) t -> p c t", p=P)[:, :, ts(g, QG)])
                if split:
                    h_ = NDCH // 2
                    for dsl in (slice(0, h_), slice(h_, NDCH)):
                        for x_sb, xr in zip(pair, srcs):
                            nc.sync.dma_start(
                                out=x_sb[:, dsl, :], in_=xr[:, dsl, :])
                else:
                    for x_sb, xr in zip(pair, srcs):
                        nc.sync.dma_start(out=x_sb, in_=xr)
                return tuple(pair)

            def proj_mms(ps, w_pair, x_pair, fc, half):
                """3-term hi/lo fp8 DoubleRow chain: w8*x8 + w8l*x8 + w8*x8l.
                Contraction pairs c of 256 rows; 3 DR matmuls each. The
                x8l term runs last so prelude chains start before the lo
                DMAs land."""
                w8, w8l = w_pair
                x8, x8l = x_pair
                cs = list(range(0, NDCH // 4) if half == 0 else (
                    range(NDCH // 4, NDCH // 2) if half == 1
                    else range(NDCH // 2)))
                ncp = NDCH // 2
                # term-major: all hi-x matmuls first so a late x8l DMA
                # never blocks the middle of a chain (PSUM accumulation
                # order is free)
                for t, (wt, xt) in enumerate(
                    ((w8, x8), (w8l, x8), (w8, x8l))
                ):
                    for c in cs:
                        d = slice(2 * c, 2 * c + 2)
                        nc.tensor.matmul(
                            ps, wt[:, d, ts(fc, P)], xt[:, d, :],
                            start=(c == 0 and t == 0),
                            stop=(c == ncp - 1 and t == 2),
                            perf_mode=DR,
                        )

            def kproj_chain(w_sb, x_sb, g, fc, half=None, state={}):
                if half in (None, 0):
                    state["ps"] = pp.tile(
                        [P, QG], f32, tag="pp", name=f"pk_{g}_{fc}"
                    )
                ps = state["ps"]
                proj_mms(ps, w_sb, x_sb, fc, half)
                if half in (None, 1):
                    nc.scalar.add(out=kT[:, fc, ts(g, QG)], in_=ps,
                                  add=bk_sb[:, fc : fc + 1])

            def qproj_chain(w_sb, x_sb, qT, g, fc, half=None, state={}):
                if half in (None, 0):
                    state["ps"] = pp.tile(
                        [P, QG], f32, tag="pp", name=f"pq_{g}_{fc}"
                    )
                ps = state["ps"]
                proj_mms(ps, w_sb, x_sb, fc, half)
                if half in (None, 1):
                    nc.scalar.add(out=qT[:, fc, :], in_=ps,
                                  add=bq_sb[:, fc : fc + 1])

            def vproj_tile(w_sb, x_sb, kt):
                tt = kt % NQT
                w8, w8l = w_sb
                x8, x8l = x_sb
                ps = pp.tile([P, DC], f32, tag="pp", name=f"pv_{kt}")
                ncp = NDCH // 2
                for c in range(ncp):
                    d = slice(2 * c, 2 * c + 2)
                    for t, (xt, wt) in enumerate(
                        ((x8, w8), (x8, w8l), (x8l, w8))
                    ):
                        nc.tensor.matmul(
                            ps, xt[:, d, ts(tt, P)], wt[:, d, :],
                            start=(c == 0 and t == 0),
                            stop=(c == ncp - 1 and t == 2),
                            perf_mode=DR,
                        )
                # no v-bias on device: softmax rows sum to 1, so bv is a
                # constant row shift of the attention output — the host adds
                # bv @ wo with the output bias. The drain is a pure copy on
                # ScalarE, keeping DVE free for the exp stream.
                nc.scalar.copy(
                    out=v_aug[:, kt, :, 0:DK],
                    in_=ps.rearrange("p (h d) -> p h d", h=HC),
                )

            ob_tiles = {}

            def outproj_chain(attnT, g, tt, eg, pool=None, ptag="pp",
                              copy_eng="vector", dma_split=False):
                pool = pool or pp
                ps = pool.tile(
                    [P, DC], f32, tag=ptag, name=f"po_{g}_{tt}_{eg}",
                )
                for fc in range(NFC):
                    nc.tensor.matmul(
                        ps, attnT[:, fc, ts(tt, P)], wo_sb[:, fc, ts(eg, DC)],
                        start=(fc == 0), stop=(fc == NFC - 1),
                    )
                # batch the two half-row stores into one [P, 1024] DMA;
                # copies ride ScalarE so DVE's exp queue stays hiccup-free
                if eg == 0:
                    ob_tiles[(g, tt)] = ocpool.tile(
                        [P, 2, DC], bf16, tag="osb", name=f"ob_{g}_{tt}"
                    )
                o_sb = ob_tiles[(g, tt)]
                if copy_eng == "scalar":
                    nc.scalar.copy(out=o_sb[:, eg, :], in_=ps)
                else:
                    nc.vector.tensor_copy(out=o_sb[:, eg, :], in_=ps)
                if dma_split:
                    # tail: fire per-half DMAs so the last copy overlaps the
                    # previous half's store
                    nc.sync.dma_start(
                        out=out[ds(g * QG + tt * P, P), ts(eg, DC)],
                        in_=o_sb[:, eg, :],
                    )
                elif eg == 1:
                    nc.sync.dma_start(
                        out=out[ds(g * QG + tt * P, P), :], in_=o_sb
                    )

            # ---- pair state: est tiles + av banks, consumed one pair later
            class PairState:
                def __init__(self, g, hp):
                    self.g, self.hp = g, hp
                    self.ests = {}   # h -> list of 8 est tiles [P, 2, QG]
                    self.av = None   # 2x [P, 2, 2, VW] f32 psum (1 bank each)

            def attv_slice(ps_, s):
                """att@V consuming est[s] (key tiles 2s, 2s+1). The softmax
                denominator rides along as column DK (the v_aug ones column)
                so there is one 65-col matmul per (kt, qt, hh) instead of a
                64-col + a 1-col Z matmul; the av accumulator splits into
                two banks of 2 q-tiles each."""
                g, hp = ps_.g, ps_.hp
                if s == 0:
                    ps_.av = [
                        avp.tile([P, 2, 2, VW], f32, tag=f"av{b}",
                                 name=f"av_{g}_{hp}_{b}")
                        for b in range(2)
                    ]
                last = NKT // 2 - 1
                for kk in range(2):
                    kt = 2 * s + kk
                    for qt in range(NQT):
                        for hh in range(2):
                            h = 2 * hp + hh
                            est = ps_.ests[h][s]
                            stat = est[:, kk, ts(qt, P)]
                            first = (s == 0 and kk == 0 and qt % 2 == 0
                                     and hh == 0)
                            lastm = (s == last and kk == 1 and qt % 2 == 1
                                     and hh == 1)
                            nc.tensor.matmul(
                                ps_.av[qt // 2][:, qt % 2, hh, :], stat,
                                v_aug[:, kt, h, :],
                                start=first, stop=lastm,
                            )

            def finish_pair(ps_, o_sb_tiles, qts=None):
                """reciprocal + normalize for a finished pair.

                qts: restrict the normalize to these q-tiles (tail
                pipelining); reciprocal runs only when qts is None or
                starts at qt 0."""
                g, hp = ps_.g, ps_.hp
                if qts is None or qts[0] == 0:
                    ps_.rz = spool.tile(
                        [P, NQT, 2], f32, tag="rz", name=f"rz_{g}_{hp}"
                    )
                    with nc.allow_low_precision("softmax denom reciprocal"):
                        for b in range(2):
                            nc.vector.reciprocal(
                                out=ps_.rz[:, 2 * b : 2 * b + 2, :],
                                in_=ps_.av[b][:, :, :, DK],
                            )
                o_sb = o_sb_tiles[g]
                if qts is None:
                    # the whole normalize must drain before the next pair's
                    # attV reuses the av banks (ring depth 1), so the
                    # forced slot-0 burst is split across both engines:
                    # qt2/3 as DVE TTs, qt0/1 as per-partition-scale Copy
                    # activations on ScalarE
                    for qt in (2, 3):
                        nc.vector.tensor_tensor(
                            out=o_sb[:, qt, 2 * hp : 2 * hp + 2, :],
                            in0=ps_.av[qt // 2][:, qt % 2, :, 0:DK],
                            in1=ps_.rz[:, qt, :].unsqueeze(-1).broadcast_to(
                                [P, 2, DK]),
                            op=ALU.mult,
                        )
                    for qt in (0, 1):
                        for hh in range(2):
                            nc.scalar.activation(
                                out=o_sb[:, qt, 2 * hp + hh, :],
                                in_=ps_.av[qt // 2][:, qt % 2, hh, 0:DK],
                                func=AF.Copy,
                                scale=ps_.rz[:, qt, hh : hh + 1],
                            )
                else:
                    for qt in qts:
                        nc.vector.tensor_tensor(
                            out=o_sb[:, qt, 2 * hp : 2 * hp + 2, :],
                            in0=ps_.av[qt // 2][:, qt % 2, :, 0:DK],
                            in1=ps_.rz[:, qt, :].unsqueeze(-1).broadcast_to(
                                [P, 2, DK]),
                            op=ALU.mult,
                        )

            def transposes(g, o_sb_tiles, attnT):
                o_sb = o_sb_tiles[g]
                for qt in range(NQT):
                    for fc in range(NFC):
                        nc.sync.dma_start_transpose(
                            out=attnT[:, fc, ts(qt, P)],
                            in_=o_sb[:, qt, 2 * fc : 2 * fc + 2, :],
                        )

            # =========== prelude ===========
            # HWDGE queue (~625ns/DMA) and DMA bandwidth (~2.84us/MB) are
            # both serial, so DMAs are ordered by consumer deadline: the
            # fc0 weight columns + xk0/xq0 feed the first kproj/qproj
            # chains (~2.5MB -> first score at ~10us); the rest queue in
            # the order the spliced chains consume them.
            wk_sb, wk_rest = load_w_fc(wk, "w_k", "w")
            xk_sbs = [load_x(xkT, 0, "x_k_0", tag="xk", bufs=4)]
            small_loads()
            wq_sb, wq_rest = load_w_fc(wq, "w_q", "wq", bufs=1)
            xq_tiles = {0: load_x(xqT, 0, "x_q_0", tag="xq", bufs=2)}
            xk_sbs.append(load_x(xkT, 1, "x_k_1", tag="xk", bufs=4))
            xk_sbs.append(load_x(xkT, 2, "x_k_2", tag="xk", bufs=4))
            xk_sbs.append(load_x(xkT, 3, "x_k_3", tag="xk", bufs=4))
            wq_rest()
            wk_rest()
            wv_sb = load_w(wv, "w_v")
            xv_tiles = {0: load_x(xvT, 0, "x_v_0", tag="xv", bufs=2)}

            kproj_chain(wk_sb, xk_sbs[0], 0, 0, half=0)
            kproj_chain(wk_sb, xk_sbs[0], 0, 0, half=1)
            qst = {0: qpool.tile([P, NFC, QG], bf16, tag="qT", name="qT_0")}
            qproj_chain(wq_sb, xq_tiles[0], qst[0], 0, 0, half=0)
            qproj_chain(wq_sb, xq_tiles[0], qst[0], 0, 0, half=1)

            # =========== splice schedule ===========
            # pair index p = 4*g + hp runs score loop slots 0..7; sched[p][s]
            # is a list of thunks emitted before slot s's score matmuls.
            sched = {p: {s: [] for s in range(8)} for p in range(16)}

            def at(p, s, fn):
                sched[p][s].append(fn)

            # kproj: fc=0 for kg>=1 early in pair 0 (slots track xk DMA
            # arrival); pair (0,hp) reads kT chunk hp for all kt: chunk fc
            # must be fully projected (all 4 kg) before pair (0,fc) starts.
            for kg, s_ in [(1, 1), (2, 3), (3, 5)]:
                at(0, s_, lambda kg=kg: kproj_chain(wk_sb, xk_sbs[kg], kg, 0))
            kproj_slots = {1: [(0, 6), (0, 7), (1, 1), (1, 3)],
                           2: [(1, 4), (1, 6), (2, 0), (2, 2)],
                           3: [(2, 3), (2, 5), (3, 0), (3, 2)]}
            for fc in range(1, 4):
                for kg, (p_, s_) in enumerate(kproj_slots[fc]):
                    at(p_, s_, lambda kg=kg, fc=fc: kproj_chain(
                        wk_sb, xk_sbs[kg], kg, fc))
            # wo: first consumer is outproj(g=0) at pair 5; queue after the
            # prelude loads
            at(0, 6, lambda: nc.sync.dma_start(
                out=wo_sb, in_=wo.rearrange("(c p) e -> p c e", p=P)))
            # vproj rides pair 1 (xv0 lands at ~24us); emitted before that
            # slot's attV splice (pair 1 emits sched fns first) so attV(0)
            # sees the fresh v_aug tiles. xv loads 2+ slots ahead.
            for vg, (p_, s_) in {1: (0, 1), 2: (0, 5), 3: (0, 7)}.items():
                at(p_, s_, lambda vg=vg: xv_tiles.__setitem__(
                    vg, load_x(xvT, vg, f"x_v_{vg}", tag="xv", bufs=2)))
            vq = [(1, 0, 2), (1, 1, 2), (1, 2, 2), (1, 3, 2), (1, 4, 2),
                  (1, 5, 2), (1, 6, 2), (1, 7, 2)]
            kt_next = 0
            for p_, s_, n_ in vq:
                for _ in range(n_):
                    if kt_next >= NKT:
                        break
                    kt = kt_next
                    kt_next += 1
                    at(p_, s_, lambda kt=kt: vproj_tile(
                        wv_sb, xv_tiles[kt // NQT], kt))
            # qproj for pair p+1 at pair p slots 1/5: odd slots carry the
            # spliced PE work so thin even slots never dip below the exp
            # engines' slot rate (+ xq loads 2 pairs early)
            for p in range(15):
                g1, fc1 = divmod(p + 1, 4)
                s0, s1 = (4, 6) if p == 0 else (1, 5)
                if fc1 == 0 and g1 > 0:
                    at(p - 2 if p >= 2 else 0, 1, lambda g1=g1: xq_tiles.__setitem__(
                        g1, load_x(xqT, g1, f"x_q_{g1}", tag="xq", bufs=2)))
                    at(p, s0, lambda g1=g1: (
                        qst.__setitem__(g1, qpool.tile(
                            [P, NFC, QG], bf16, tag="qT", name=f"qT_{g1}")),
                        qproj_chain(wq_sb, xq_tiles[g1], qst[g1], g1, 0,
                                    half=0))[-1])
                    at(p, s1, lambda g1=g1: qproj_chain(
                        wq_sb, xq_tiles[g1], qst[g1], g1, 0, half=1))
                else:
                    at(p, s0, lambda g1=g1, fc1=fc1: qproj_chain(
                        wq_sb, xq_tiles[g1], qst[g1], g1, fc1, half=0))
                    at(p, s1, lambda g1=g1, fc1=fc1: qproj_chain(
                        wq_sb, xq_tiles[g1], qst[g1], g1, fc1, half=1))
            # outproj(g) chains spliced into pairs of group g+1, odd slots
            op_slots = [(1, 1), (1, 3), (1, 5), (2, 1), (2, 3), (2, 5),
                        (3, 1), (3, 3)]
            attnT_holder = {}
            for g in range(3):
                for i, (hp_, s_) in enumerate(op_slots):
                    tt, eg = divmod(i, 2)
                    at(4 * (g + 1) + hp_, s_, lambda g=g, tt=tt, eg=eg: outproj_chain(
                        attnT_holder[g], g, tt, eg))

            # =========== main loop ===========
            o_sb_tiles = {}
            prev_pair = None   # PairState consumed by current pair's splices
            done_pair = None   # PairState whose attV completed last pair
            # (its finish_pair runs at the START of this pair so the DVE
            # queue never parks on unmet deps — DVE is in-order)

            for p in range(16):
                g, hp = divmod(p, 4)
                if g not in o_sb_tiles:
                    o_sb_tiles[g] = ospool.tile(
                        [P, NQT, HC, DK], bf16, tag="osb2", name=f"o_{g}"
                    )
                cur = PairState(g, hp)
                qT = qst[g]
                for kt2 in range(NKT // 2):
                    if kt2 == 0 and done_pair is not None:
                        finish_pair(done_pair, o_sb_tiles)
                        if done_pair.hp == NHP - 1:
                            gg = done_pair.g
                            attnT_holder[gg] = atpool.tile(
                                [P, NFC, QG], bf16, tag="attnT", name=f"aT_{gg}"
                            )
                            transposes(gg, o_sb_tiles, attnT_holder[gg])
                        done_pair = None
                    def emit_splices():
                        if prev_pair is not None:
                            attv_slice(prev_pair, kt2)
                        for fn in sched[p][kt2]:
                            fn()

                    def emit_splices_vfirst():
                        # pair 1: vproj fns must precede the attV splice that
                        # reads their v_aug tiles (Tile deps are
                        # emission-ordered)
                        for fn in sched[p][kt2]:
                            fn()
                        if prev_pair is not None:
                            attv_slice(prev_pair, kt2)

                    def emit_scores():
                        # one 1-bank PSUM tile per (head, kt): each head's
                        # 2-buf ring turns over within the slot (kk0's exp
                        # completes while kk1's scores run), so the next
                        # slot's scores never wait a full exp round-trip —
                        # breaks the per-slot PE<->exp lockstep at the same
                        # 4-bank PSUM cost
                        sts = {}
                        ests = {}
                        for hh in range(2):
                            h = 2 * hp + hh
                            e = epool.tile(
                                [P, 2, QG], bf16, tag="est",
                                name=f"est_{g}_{h}_{kt2}"
                            )
                            ests[h] = e
                            cur.ests.setdefault(h, []).append(e)
                        for kk in range(2):
                            kt = 2 * kt2 + kk
                            for hh in range(2):
                                h = 2 * hp + hh
                                r0 = hh * DK
                                sts[(h, kk)] = st_pool.tile(
                                    [P, QG], f32, tag=f"st{hh}",
                                    name=f"st_{g}_{h}_{kt2}_{kk}"
                                )
                                nc.tensor.matmul(
                                    sts[(h, kk)],
                                    kT[r0 : r0 + DK, hp, ts(kt, P)],
                                    qT[r0 : r0 + DK, hp, :],
                                    start=True, stop=True,
                                    tile_position=(r0, 0),
                                )
                            for hh in range(2):
                                h = 2 * hp + hh
                                e = ests[h]
                                if hh == 1:
                                    # Schraudolph exp on DVE: bf16 bits of
                                    # exp(s*INV_SCALE) ~= trunc(A*s + B)
                                    # (max rel err ~3%, softmax-cancelled)
                                    nc.vector.tensor_scalar(
                                        out=e.bitcast(mybir.dt.int16)[:, kk, :],
                                        in0=sts[(h, kk)],
                                        scalar1=SCH_A, scalar2=SCH_B,
                                        op0=ALU.mult, op1=ALU.add,
                                    )
                                else:
                                    nc.scalar.activation(
                                        out=e[:, kk, :], in_=sts[(h, kk)],
                                        func=AF.Exp, scale=INV_SCALE
                                    )

                    emit_scores()
                    if p == 1:
                        emit_splices_vfirst()
                    elif p == 15:
                        # last pair: front-load pair-14's attV (2 slices per
                        # slot, ests are all ready), finish pair 14 at slot 4
                        # to free the av/z banks, then start pair-15's own
                        # attV in slots 5-7 — the tail keeps only slices 6-7
                        for fn in sched[p][kt2]:
                            fn()
                        if kt2 < 4:
                            attv_slice(prev_pair, 2 * kt2)
                            attv_slice(prev_pair, 2 * kt2 + 1)
                        elif kt2 == 4:
                            finish_pair(prev_pair, o_sb_tiles)
                        else:
                            attv_slice(cur, 2 * (kt2 - 5))
                            attv_slice(cur, 2 * (kt2 - 5) + 1)
                    else:
                        emit_splices()
                # previous pair's attV is complete; finish it at the start
                # of the next pair (deps met there, no DVE queue parking)
                done_pair = prev_pair
                prev_pair = cur

            # =========== tail: last attV slices + outproj of group 3 ====
            # pair 14 was finished inside pair 15 (slot 4) and pair-15's
            # attV slices 0-5 ran in slots 5-7; only slices 6-7 remain.
            # per-qt pipelining: as soon as qt's normalize lands, its
            # transposes, outproj chains and output DMA flow while the PE
            # works the next qt.
            attv_slice(prev_pair, 6)
            attv_slice(prev_pair, 7)
            attnT_holder[3] = atpool.tile(
                [P, NFC, QG], bf16, tag="attnT", name="aT_3"
            )
            aT3 = attnT_holder[3]
            o_sb3 = o_sb_tiles[3]
            # group-3 transposes run on the PE (53ns each, vs 625ns serial
            # HWDGE slots that paced the old tail): fc0-2 (pairs 12-14, long
            # normalized) transpose for all q-tiles right away into the
            # freed score banks, DVE copies them to SBUF; fc3 follows each
            # q-tile's normalize.
            tp_tiles = {}
            for qt in range(NQT):
                tp = st_pool.tile([P, NFC, P], bf16, tag=f"st{qt % 2}",
                                  name=f"tp_{qt}")
                tp_tiles[qt] = tp
                for fc in range(3):
                    nc.tensor.transpose(
                        tp[:, fc, :], o_sb3[:, qt, 2 * fc : 2 * fc + 2, :],
                        ident[:, :],
                    )
                nc.vector.tensor_copy(
                    out=aT3[:, 0:3, ts(qt, P)], in_=tp[:, 0:3, :]
                )
            for qt in range(NQT):
                finish_pair(prev_pair, o_sb_tiles, qts=[qt])
                tp = tp_tiles[qt]
                nc.tensor.transpose(
                    tp[:, 3, :], o_sb3[:, qt, 6:8, :], ident[:, :]
                )
                nc.scalar.copy(out=aT3[:, 3, ts(qt, P)], in_=tp[:, 3, :])
                for eg in range(2):
                    # chains mostly ride the pp ring; two chains borrow the
                    # av banks once their halves are fully normalized (av0
                    # after qt1's finish, av1 after qt3's). copies split
                    # across DVE (idle at tail) and ScalarE
                    idx = qt * 2 + eg
                    pool_, ptag_ = ((avp, "av0") if idx == 2 else
                                    (avp, "av1") if idx == 6 else (pp, "pp"))
                    outproj_chain(
                        attnT_holder[3], 3, qt, eg,
                        pool=pool_, ptag=ptag_,
                        copy_eng=("vector" if eg == 0 else "scalar"),
                        dma_split=(qt == NQT - 1),
                    )

    nc.compile()
    return nc


def _get_nc(debug=False):
    if "nc" not in _CACHE:
        _CACHE["nc"] = _build()
    return _CACHE["nc"]


def _tf32(a):
    """Round fp32 to the TF32 grid (10-bit mantissa, round-to-nearest-even)."""
    u = np.ascontiguousarray(a, dtype=np.float32).view(np.uint32)
    u = (u + np.uint32(0xFFF) + ((u >> np.uint32(13)) & np.uint32(1))) & np.uint32(
        0xFFFFE000
    )
    return u.view(np.float32)


def _bf16(a):
    import ml_dtypes

    return np.ascontiguousarray(a, dtype=np.float32).astype(ml_dtypes.bfloat16)


def _make_in_maps(inputs):
    q = np.asarray(inputs["query"], dtype=np.float32)
    k = np.asarray(inputs["key"], dtype=np.float32)
    v = np.asarray(inputs["value"], dtype=np.float32)
    wq = np.asarray(inputs["wq"], dtype=np.float32)
    wk = np.asarray(inputs["wk"], dtype=np.float32)
    wv = np.asarray(inputs["wv"], dtype=np.float32)
    wo = np.asarray(inputs["wo"], dtype=np.float32)
    bq = np.asarray(inputs["bq"], dtype=np.float32)
    bk = np.asarray(inputs["bk"], dtype=np.float32)
    bv = np.asarray(inputs["bv"], dtype=np.float32)

    import ml_dtypes

    def _hl(a):
        hi = np.ascontiguousarray(a, dtype=np.float32).astype(
            ml_dtypes.float8_e4m3)
        lo = (a - hi.astype(np.float32)).astype(ml_dtypes.float8_e4m3)
        return hi, lo

    WS = 32.0  # fp8 weight pre-scale (undone via exp scale / ones column)
    xT = [(_hl(q[b].T), _hl(k[b].T), _hl(v[b].T)) for b in range(B)]

    def _fcmajor(w):
        # [D, DC] -> [NFC, P(partition), NDCH, P(col)]: per-partition 1KB
        # contiguous blocks for full-rate DMA of fc-column ranges
        return np.ascontiguousarray(
            w.reshape(NDCH, P, NFC, P).transpose(2, 1, 0, 3))

    in_maps = []
    for c in range(NCORES):
        b, g = divmod(c, 2)
        sl = slice(g * DC, (g + 1) * DC)
        wq8, wq8l = (_fcmajor(a) for a in _hl(wq[:, sl] * WS))
        wk8, wk8l = (_fcmajor(a) for a in _hl(wk[:, sl] * WS))
        wv8, wv8l = _hl(wv[:, sl] * WS)
        in_maps.append(
            {
                "xq8": xT[b][0][0], "xq8l": xT[b][0][1],
                "xk8": xT[b][1][0], "xk8l": xT[b][1][1],
                "xv8": xT[b][2][0], "xv8l": xT[b][2][1],
                "wq8": wq8, "wq8l": wq8l,
                "wk8": wk8, "wk8l": wk8l,
                "wv8": wv8, "wv8l": wv8l,
                "wo": _bf16(wo[sl, :]),
                "bq": np.ascontiguousarray(bq[sl] * WS),
                "bk": np.ascontiguousarray(bk[sl] * WS),
            }
        )
    return in_maps


def run(inputs, **kwargs):
    """Run the kernel; returns (full_output, BassKernelResults)."""
    from concourse.bass_utils import run_bass_kernel_spmd

    kwargs.pop("debug", None)
    nc = _get_nc()
    in_maps = _make_in_maps(inputs)
    res = run_bass_kernel_spmd(nc, in_maps, core_ids=list(range(NCORES)), **kwargs)
    bo = np.asarray(inputs["bo"], dtype=np.float32)
    # v-bias folded out of the device kernel: softmax rows sum to 1, so
    # att @ (V + 1 bv^T) = att @ V + bv^T, and bv rides the host bias as
    # bv @ wo
    bvwo = np.asarray(inputs["bv"], np.float32) @ np.asarray(
        inputs["wo"], np.float32)
    final = np.empty((B, S, D), np.float32)
    for b in range(B):
        final[b] = (
            res.results[2 * b]["out"].astype(np.float32)
            + res.results[2 * b + 1]["out"].astype(np.float32)
            + bo
            + bvwo
        )
    return final, res


def kernel(**inputs):
    return run(inputs)[0]

